# revision 73
# baseline (speedup 1.0000x reference)
"""Trainium2 Bass kernel for nn_BlockModel_82678120448388.

Model: per (batch, head): 8x8 transition matrices from an MLP (normalized),
values from a second MLP, then a linear recurrence s_t = A_t s_{t-1} + v_t
over seq=2048.

Sharding: 8 cores = 4 batches x 2 head-halves (32 heads each). Weights
replicated / row-sliced on host; full inputs in, full output out.

Scan: chunked scan (K=16 chunks x C=128). Phase 1 composes the per-chunk
[T|u] prefix chain in bf16 (mult + add-tree on DVE, all operands packed for
the 2x mode), interleaved under the MLP matmuls, spilling each prefix to
DRAM. Phase B combines chunk totals. Phase C applies s_r = T_r s_init + u_r
from the spilled prefixes -- fully parallel, no dependency chain.
"""

import numpy as np
import ml_dtypes
from contextlib import ExitStack

import concourse.bass as bass
import concourse.bacc as bacc
import concourse.tile as tile
from concourse import mybir

F32 = mybir.dt.float32
BF16 = mybir.dt.bfloat16
AF = mybir.ActivationFunctionType
ALU = mybir.AluOpType

BS, SEQ, EMB, BD = 4, 2048, 512, 8
H = EMB // BD      # 64 global heads
HL = 32            # heads per core
NF = HL * BD * BD  # 2048 blk feats per core
VF = HL * BD       # 256 v feats per core
AVF = NF + VF      # 2304 combined A+v row
HID = EMB * BD     # 4096
P = 128
JW = BD + 1        # [T|u] column count

N_CORES = 8


def build_nc(TOK=SEQ, K=16, p1_steps=None, pc_steps=None, nq_steps=None):
    """Per-core Bass module. TOK tokens, K chunks (chunk len C=TOK//K)."""
    C = TOK // K
    QT = min(256, TOK)     # L1 token-chunk
    NQ = TOK // QT
    TPQ = QT // P          # tok-tiles per q
    NHO = P // K           # head-groups per chunk on partitions (8 for K=16)
    NHR = HL // NHO        # heads per group in free dim (4)
    HRI = NHR * BD         # 32
    TUP = NHR * BD * JW    # 288 = per-partition [T|u]^T size (hr, m9, i8)
    GR = NHR * BD * BD     # 256 = A feats per (ho) group
    GRV = GR + NHR * BD    # 288 = A+v feats per (ho) group

    assert TOK % QT == 0 and QT % P == 0 and P % K == 0

    nc = bacc.Bacc("TRN2", target_bir_lowering=False, debug=False)

    xT = nc.dram_tensor("xT", [EMB, TOK], BF16, kind="ExternalInput")
    w1 = nc.dram_tensor("w1", [EMB, HID], BF16, kind="ExternalInput")
    b1 = nc.dram_tensor("b1", [HID, 1], F32, kind="ExternalInput")
    w2 = nc.dram_tensor("w2", [HID, NF], BF16, kind="ExternalInput")
    b2 = nc.dram_tensor("b2", [1, NF], BF16, kind="ExternalInput")
    v1 = nc.dram_tensor("v1", [EMB, EMB], BF16, kind="ExternalInput")
    c1 = nc.dram_tensor("c1", [EMB, 1], F32, kind="ExternalInput")
    v2 = nc.dram_tensor("v2", [EMB, VF], BF16, kind="ExternalInput")
    c2 = nc.dram_tensor("c2", [1, VF], BF16, kind="ExternalInput")
    a0 = nc.dram_tensor("a0", [NHO, HRI], F32, kind="ExternalInput")
    # native scan layout [(c,ho), (r, hr, i)]; host reindexes to [t, vf]
    out = nc.dram_tensor("out", [P, C * HRI], F32, kind="ExternalOutput")

    # spilled per-token prefixes [T|u]^T, r-minor: [P, C, TUP]
    tu_dram = nc.dram_tensor("tu_scratch", [P, C * TUP], BF16)

    with ExitStack() as ctx:
        tc = ctx.enter_context(tile.TileContext(nc))
        cpool = ctx.enter_context(tc.tile_pool(name="consts", bufs=1))
        wpool = ctx.enter_context(tc.tile_pool(name="weights", bufs=1))
        xpool = ctx.enter_context(tc.tile_pool(name="xstream", bufs=2))
        hpool = ctx.enter_context(tc.tile_pool(name="hidden", bufs=2))
        w2pool = ctx.enter_context(tc.tile_pool(name="w2stream", bufs=4))
        l1ps = ctx.enter_context(tc.tile_pool(name="l1ps", bufs=2, space="PSUM"))
        l2ps = ctx.enter_context(tc.tile_pool(name="l2ps", bufs=2 * TPQ, space="PSUM"))
        vps = ctx.enter_context(tc.tile_pool(name="vps", bufs=2, space="PSUM"))
        blkpool = ctx.enter_context(tc.tile_pool(name="blk", bufs=TPQ + 1))
        abpool = ctx.enter_context(tc.tile_pool(name="ab", bufs=2))
        sqpool = ctx.enter_context(tc.tile_pool(name="sq", bufs=2))
        smpool = ctx.enter_context(tc.tile_pool(name="small", bufs=2))
        agpool = ctx.enter_context(tc.tile_pool(name="agather", bufs=4))
        mopool = ctx.enter_context(tc.tile_pool(name="multout", bufs=2))
        tupool = ctx.enter_context(tc.tile_pool(name="tu", bufs=3))
        tbpool = ctx.enter_context(tc.tile_pool(name="tub", bufs=3))
        ctpool = ctx.enter_context(tc.tile_pool(name="ctree", bufs=1))
        scpool = ctx.enter_context(tc.tile_pool(name="scan", bufs=1))

        # ---- constants / weights ----
        ones_s = cpool.tile([1, P], BF16, tag="ones")
        nc.vector.memset(ones_s[:], 1.0)
        b1_s = cpool.tile([P, HID // P], F32, tag="b1")
        nc.sync.dma_start(b1_s[:], b1[:].rearrange("(m p) one -> p (m one)", p=P))
        c1_s = cpool.tile([P, EMB // P], F32, tag="c1")
        nc.sync.dma_start(c1_s[:], c1[:].rearrange("(m p) one -> p (m one)", p=P))
        b2_s = cpool.tile([1, NF], BF16, tag="b2")
        nc.sync.dma_start(b2_s[:], b2[:])
        c2_s = cpool.tile([1, VF], BF16, tag="c2")
        nc.sync.dma_start(c2_s[:], c2[:])
        a0_s = cpool.tile([NHO, HRI], F32, tag="a0")
        nc.sync.dma_start(a0_s[:], a0[:])

        w1_s = wpool.tile([P, 4, HID], BF16, tag="w1")
        v1_s = wpool.tile([P, 4, EMB], BF16, tag="v1")
        v2_s = wpool.tile([P, 4, VF], BF16, tag="v2")

        def load_weights():
            # w1 in m-blocks so the first L1 matmul starts after one block
            for b in range(8):
                nc.sync.dma_start(
                    bass.AP(w1_s.tensor, w1_s[:].offset + b * 512,
                            [[4 * HID, P], [HID, 4], [1, 512]]),
                    bass.AP(w1, b * 512,
                            [[HID, P], [P * HID, 4], [1, 512]]))
            nc.sync.dma_start(v1_s[:], v1[:].rearrange("(k p) m -> p k m", p=P))
            nc.sync.dma_start(v2_s[:], v2[:].rearrange("(k p) n -> p k n", p=P))

        # ================= scan helpers =================
        # av_dram row tau*128 + c*8 + j holds token c*C + 8*tau + j.
        # Phase 1 partition = (chunk c, ho); tuT[(c,ho), (hr, m9, i8)] =
        # [T|u][i, m] (transposed storage; u col at m=8, offset 64+i).

        def rowbase(r):
            tau, j = r // 8, r % 8
            return tau * P + j

        tu_box = {}

        def g_av(r, ab):
            # SBUF->SBUF redistribution: src partitions c*8+j of ab(tau)
            j = r % 8
            ag = agpool.tile([P, GRV], BF16, tag="ag", name=f"ag{r}")
            nc.sync.dma_start(ag[:], bass.AP(
                ab.tensor, ab[:].offset + j * AVF,
                [[8 * AVF, K], [GRV, NHO], [1, GRV]]))
            return ag

        def spill_tu(r, tu):
            nc.sync.dma_start(
                bass.AP(tu_dram, r * TUP, [[C * TUP, P], [1, TUP]]),
                tu[:])

        def phase1_init(ag):
            # tuT_0[m, i] = A_0[i, m] ; ag A block row-major (hr, i, m)
            tu = tupool.tile([P, TUP], BF16, tag="tu", name="tu0")
            nc.vector.tensor_copy(
                bass.AP(tu.tensor, tu[:].offset,
                        [[TUP, P], [BD * JW, NHR], [BD, BD], [1, BD]]),
                bass.AP(ag.tensor, ag[:].offset,
                        [[GRV, P], [BD * BD, NHR], [1, BD], [BD, BD]]))
            nc.vector.tensor_copy(
                bass.AP(tu.tensor, tu[:].offset + BD * BD,
                        [[TUP, P], [BD * JW, NHR], [1, BD]]),
                bass.AP(ag.tensor, ag[:].offset + GR,
                        [[GRV, P], [BD, NHR], [1, BD]]))
            tu_box['tu'] = tu
            spill_tu(0, tu)

        def phase1_step(r, ag):
            # mo[hr, i, m, k] = A_r[i, k] * Told[k, m]; all packed bf16
            tu = tu_box['tu']
            mo = mopool.tile([P, NHR * BD * JW * BD], BF16, tag="mo",
                             name=f"mo{r}")
            # iteration (hr, i, m, k); per-operand dims merged to 3 free
            nc.vector.tensor_tensor(
                bass.AP(mo.tensor, mo[:].offset,
                        [[NHR * BD * JW * BD, P], [JW * BD, NHR * BD],
                         [BD, JW], [1, BD]]),
                bass.AP(ag.tensor, ag[:].offset,
                        [[GRV, P], [BD, NHR * BD], [0, JW], [1, BD]]),
                bass.AP(tu.tensor, tu[:].offset,
                        [[TUP, P], [BD * JW, NHR], [0, BD], [1, BD * JW]]),
                ALU.mult)
            # add-tree over k (innermost): 8 -> 4 -> 2 -> 1
            t1 = mopool.tile([P, NHR * BD * JW * 4], BF16, tag="t1",
                             name=f"t1_{r}")
            nc.vector.tensor_tensor(
                t1[:], bass.AP(mo.tensor, mo[:].offset,
                               [[NHR * BD * JW * BD, P], [BD, NHR * BD * JW],
                                [1, 4]]),
                bass.AP(mo.tensor, mo[:].offset + 4,
                        [[NHR * BD * JW * BD, P], [BD, NHR * BD * JW], [1, 4]]),
                ALU.add)
            t2 = mopool.tile([P, NHR * BD * JW * 2], BF16, tag="t2",
                             name=f"t2_{r}")
            nc.vector.tensor_tensor(
                t2[:], bass.AP(t1.tensor, t1[:].offset,
                               [[NHR * BD * JW * 4, P], [4, NHR * BD * JW],
                                [1, 2]]),
                bass.AP(t1.tensor, t1[:].offset + 2,
                        [[NHR * BD * JW * 4, P], [4, NHR * BD * JW], [1, 2]]),
                ALU.add)
            # final add writes transposed: tun[(hr, m, i)] = T_new[i, m]
            # t2 elem (hr, i, m, k) at hr*144 + i*18 + m*2 + k
            tun = tupool.tile([P, TUP], BF16, tag="tu", name=f"tu{r}")
            nc.vector.tensor_tensor(
                bass.AP(tun.tensor, tun[:].offset,
                        [[TUP, P], [BD * JW, NHR], [BD, JW], [1, BD]]),
                bass.AP(t2.tensor, t2[:].offset,
                        [[NHR * BD * JW * 2, P], [JW * BD * 2, NHR],
                         [2, JW], [JW * 2, BD]]),
                bass.AP(t2.tensor, t2[:].offset + 1,
                        [[NHR * BD * JW * 2, P], [JW * BD * 2, NHR],
                         [2, JW], [JW * 2, BD]]),
                ALU.add)
            # u += v_r  (u col at m=8: offset 64 + i; v in ag at GR + hr*8+i)
            nc.vector.tensor_tensor(
                bass.AP(tun.tensor, tun[:].offset + BD * BD,
                        [[TUP, P], [BD * JW, NHR], [1, BD]]),
                bass.AP(tun.tensor, tun[:].offset + BD * BD,
                        [[TUP, P], [BD * JW, NHR], [1, BD]]),
                bass.AP(ag.tensor, ag[:].offset + GR,
                        [[GRV, P], [BD, NHR], [1, BD]]),
                ALU.add)
            tu_box['tu'] = tun
            spill_tu(r, tun)

        # ================= stage A (+ interleaved phase 1) =================
        def load_x(q):
            RPQ = TPQ * 8
            xq = xpool.tile([P, 4, QT], BF16, tag="xq", name=f"xq{q}")
            for ttq in range(TPQ):
                # tile tau = q*TPQ+ttq: tokens c*C + 8*tau + j, col order (c, j)
                for k in range(4):
                    nc.sync.dma_start(
                        xq[:, k, bass.ts(ttq, P)],
                        bass.AP(xT, k * P * TOK + q * RPQ + ttq * 8,
                                [[TOK, P], [C, K], [1, 8]]))
            return xq

        xq0 = load_x(0)
        load_weights()

        def stage_l1(q, xq=None):
            """first-layer MLPs for q; returns (hid_t, hv_t)."""
            if xq is None:
                xq = load_x(q)
            hid_t = hpool.tile([P, HID // P, QT], BF16, tag="hid",
                               name=f"hid{q}")
            for m in range(HID // P):
                ps = l1ps.tile([P, QT], F32, tag="l1")
                for k in range(4):
                    nc.tensor.matmul(ps[:], w1_s[:, k, bass.ts(m, P)],
                                     xq[:, k, :], start=(k == 0), stop=(k == 3))
                nc.scalar.activation(hid_t[:, m, :], ps[:], AF.Relu,
                                     bias=b1_s[:, m:m + 1])
            hv_t = hpool.tile([P, 4, QT], BF16, tag="hv", name=f"hv{q}")
            for m in range(4):
                ps = l1ps.tile([P, QT], F32, tag="l1")
                for k in range(4):
                    nc.tensor.matmul(ps[:], v1_s[:, k, bass.ts(m, P)], xq[:, k, :],
                                     start=(k == 0), stop=(k == 3))
                nc.scalar.activation(hv_t[:, m, :], ps[:], AF.Relu,
                                     bias=c1_s[:, m:m + 1])
            return hid_t, hv_t

        l1_box = {0: stage_l1(0, xq0)}
        for q in range(NQ if nq_steps is None else nq_steps):
            hid_t, hv_t = l1_box.pop(q)

            # ---- L2: token-major blk, W2 streamed 4 k-slices per DMA ----
            blks = [blkpool.tile([P, NF], BF16, tag="blk", name=f"blk{q}_{i}") for i in range(TPQ)]
            for n in range(NF // 512):
                pss = [l2ps.tile([P, 512], F32, tag="l2", name=f"l2ps{q}_{n}_{i}") for i in range(TPQ)]
                for ttq in range(TPQ):
                    nc.tensor.matmul(pss[ttq][:], ones_s[:1, :],
                                     b2_s[:1, bass.ts(n, 512)], start=True, stop=False)
                for kk in range(HID // P // 4):
                    w2s = w2pool.tile([P, 4, 512], BF16, tag="w2s")
                    nc.sync.dma_start(
                        w2s[:], w2[bass.ds(kk * 4 * P, 4 * P), bass.ts(n, 512)]
                        .rearrange("(k p) n -> p k n", p=P))
                    for k4 in range(4):
                        k = kk * 4 + k4
                        for ttq in range(TPQ):
                            nc.tensor.matmul(pss[ttq][:], hid_t[:, k, bass.ts(ttq, P)],
                                             w2s[:, k4, :], start=False,
                                             stop=(k == HID // P - 1))
                for ttq in range(TPQ):
                    nc.scalar.activation(blks[ttq][:, bass.ts(n, 512)], pss[ttq][:],
                                         AF.Identity)

            # hoist next q's L1 so its relus run before this q's norm chain
            if q + 1 < NQ:
                l1_box[q + 1] = stage_l1(q + 1)

            # ---- v2 + normalization; ACT calls batched by function ----
            abs_ = [abpool.tile([P, AVF], BF16, tag="ab", name=f"ab{q}_{i}")
                    for i in range(TPQ)]
            for ttq in range(TPQ):
                psv = vps.tile([P, VF], F32, tag="v", name=f"v{q}_{ttq}")
                nc.tensor.matmul(psv[:], ones_s[:1, :], c2_s[:1, :],
                                 start=True, stop=False)
                for k in range(4):
                    nc.tensor.matmul(psv[:], hv_t[:, k, bass.ts(ttq, P)],
                                     v2_s[:, k, :], start=False, stop=(k == 3))
                # v into ab at (ho*288 + 256 + hr*8 + i)
                nc.scalar.activation(
                    bass.AP(abs_[ttq].tensor, abs_[ttq][:].offset + GR,
                            [[AVF, P], [GRV, NHO], [1, NHR * BD]]),
                    psv[:], AF.Identity)
            pwfs = [sqpool.tile([P, NF], F32, tag="pwf", name=f"pwf{q}_{i}")
                    for i in range(TPQ)]
            for ttq in range(TPQ):
                # square on DVE (bf16 packed, 2x) to shorten the ACT chain;
                # staged in ab's A-region (overwritten by the A-write below)
                nc.vector.tensor_tensor(
                    bass.AP(abs_[ttq].tensor, abs_[ttq][:].offset,
                            [[AVF, P], [GRV, NHO], [1, GR]]),
                    bass.AP(blks[ttq].tensor, blks[ttq][:].offset,
                            [[NF, P], [GR, NHO], [1, GR]]),
                    bass.AP(blks[ttq].tensor, blks[ttq][:].offset,
                            [[NF, P], [GR, NHO], [1, GR]]),
                    ALU.mult)
            for ttq in range(TPQ):
                nc.scalar.activation(
                    pwfs[ttq][:],
                    bass.AP(abs_[ttq].tensor, abs_[ttq][:].offset,
                            [[AVF, P], [GRV, NHO], [1, GR]]),
                    AF.Ln)
            for ttq in range(TPQ):
                nc.scalar.activation(pwfs[ttq][:], pwfs[ttq][:], AF.Exp,
                                     scale=0.6)
            dms = []
            for ttq in range(TPQ):
                pw = pwfs[ttq]
                # sum over i: feat = h*64 + i*8 + k -> dims [p, h, k, i]
                pst = smpool.tile([P, HL * BD], F32, tag="pst",
                                  name=f"pst{q}_{ttq}")
                nc.vector.tensor_reduce(
                    pst[:].rearrange("p (h k) -> p h k", h=HL, k=BD),
                    bass.AP(pw.tensor, pw[:].offset,
                            [[NF, P], [64, HL], [1, BD], [8, BD]]),
                    axis=mybir.AxisListType.X, op=ALU.add)
                # max_k (sum)^(1/1.2) == (max_k sum)^(1/1.2): max in linear
                dm = smpool.tile([P, HL], F32, tag="dm", name=f"dm{q}_{ttq}")
                nc.vector.tensor_reduce(
                    dm[:].rearrange("p (h one) -> p h one", h=HL, one=1),
                    pst[:].rearrange("p (h k) -> p h k", h=HL, k=BD),
                    axis=mybir.AxisListType.X, op=ALU.max)
                dms.append(dm)
            rcs = []
            for ttq in range(TPQ):
                nc.scalar.activation(dms[ttq][:], dms[ttq][:], AF.Ln)
            for ttq in range(TPQ):
                rc = smpool.tile([P, HL], F32, tag="rc", name=f"rc{q}_{ttq}")
                # rc = dm^(-1/1.2)
                nc.scalar.activation(rc[:], dms[ttq][:], AF.Exp,
                                     scale=-1.0 / 1.2)
                rcs.append(rc)
            # ---- per tile: A-write + that tile's 8 phase-1 steps ----
            for ttq in range(TPQ):
                tt = q * TPQ + ttq
                ab, blk, rc = abs_[ttq], blks[ttq], rcs[ttq]
                # A = blk * rc (broadcast over i, k): row-major into ab
                nc.vector.tensor_tensor(
                    bass.AP(ab.tensor, ab[:].offset,
                            [[AVF, P], [GRV, NHO], [1, GR]]),
                    bass.AP(blk.tensor, blk[:].offset,
                            [[NF, P], [GR, NHO], [1, GR]]),
                    bass.AP(rc.tensor, rc[:].offset,
                            [[HL, P], [NHR, NHO], [1, NHR], [0, BD * BD]]),
                    ALU.mult)
                for r in range(tt * 8, tt * 8 + 8):
                    if p1_steps is not None and r >= p1_steps:
                        continue
                    ag = g_av(r, ab)
                    if r == 0:
                        phase1_init(ag)
                    else:
                        phase1_step(r, ag)

        # preload first phase-C prefix batches while phase B runs
        NB = 8
        def load_tub(rb):
            tub = tbpool.tile([P, NB, TUP], BF16, tag="tu4", name=f"tu4_{rb}")
            nc.sync.dma_start(tub[:], bass.AP(
                tu_dram, rb * TUP,
                [[C * TUP, P], [1, NB * TUP]]))
            return tub
        tub_q = {rb: load_tub(rb) for rb in (0, NB, 2 * NB)}

        # ---- phase B: chunk-level combine (on partitions 0:NHO) ----
        TUPK = K * TUP
        tu2 = scpool.tile([NHO, TUPK], BF16, tag="tu2")
        nc.sync.dma_start(
            bass.AP(tu2.tensor, tu2[:].offset,
                    [[TUPK, NHO], [TUP, K], [1, TUP]]),
            bass.AP(tu_dram, (C - 1) * TUP,
                    [[C * TUP, NHO], [NHO * C * TUP, K], [1, TUP]]))
        s_seq = scpool.tile([NHO, (K + 1) * HRI], F32, tag="sseq")
        nc.vector.tensor_copy(s_seq[:, 0:HRI], a0_s[:])
        for c in range(K):
            # mo3[hr, i, k] = Tc[i, k] * s[k]; Tc[i,k] at tuT offset k*8+i
            mo3 = mopool.tile([NHO, HRI * BD], F32, tag="mo3")
            nc.vector.tensor_tensor(
                bass.AP(mo3.tensor, mo3[:].offset,
                        [[HRI * BD, NHO], [BD * BD, NHR], [1, BD], [BD, BD]]),
                bass.AP(tu2.tensor, tu2[:].offset + c * TUP,
                        [[TUPK, NHO], [BD * JW, NHR], [1, BD], [BD, BD]]),
                bass.AP(s_seq.tensor, s_seq[:].offset + c * HRI,
                        [[(K + 1) * HRI, NHO], [BD, NHR], [0, BD], [1, BD]]),
                ALU.mult)
            sn3 = smpool.tile([NHO, HRI], F32, tag="sn3")
            nc.vector.tensor_reduce(
                bass.AP(sn3.tensor, sn3[:].offset, [[HRI, NHO], [1, HRI]]),
                bass.AP(mo3.tensor, mo3[:].offset,
                        [[HRI * BD, NHO], [BD, HRI], [1, BD]]),
                axis=mybir.AxisListType.X, op=ALU.add)
            nc.vector.tensor_tensor(
                bass.AP(s_seq.tensor, s_seq[:].offset + (c + 1) * HRI,
                        [[(K + 1) * HRI, NHO], [BD, NHR], [1, BD]]),
                bass.AP(sn3.tensor, sn3[:].offset, [[HRI, NHO], [BD, NHR], [1, BD]]),
                bass.AP(tu2.tensor, tu2[:].offset + c * TUP + BD * BD,
                        [[TUPK, NHO], [BD * JW, NHR], [1, BD]]),
                ALU.add)
        # relayout chunk-start states -> s_init [(c,ho), (hr,k)]
        s_init = scpool.tile([P, HRI], F32, tag="sinit")
        for c in range(K):
            nc.sync.dma_start(s_init[c * NHO:(c + 1) * NHO, :],
                              s_seq[:, c * HRI:(c + 1) * HRI])

        # ---- phase C: s_r = T_r s_init + u_r from spilled prefixes ----
        # column-major: s_r[i] = sum_m T[i, m]-col * s[m]; all bf16 packed
        s_outb = scpool.tile([P, C * HRI], BF16, tag="soutb")
        # s_mi[(slot,hr), m, i] = s_init[hr, m] replicated over slot, i
        s_mi = scpool.tile([P, NB * NHR * BD * BD], BF16, tag="smi")
        for slot in range(NB):
            nc.vector.tensor_copy(
                bass.AP(s_mi.tensor, s_mi[:].offset + slot * GR,
                        [[NB * GR, P], [BD * BD, NHR], [BD, BD], [1, BD]]),
                bass.AP(s_init.tensor, s_init[:].offset,
                        [[HRI, P], [BD, NHR], [1, BD], [0, BD]]))
        for rb in range(0, C if pc_steps is None else pc_steps, NB):
            tub = tub_q.pop(rb)
            if rb + 3 * NB < C:
                tub_q[rb + 3 * NB] = load_tub(rb + 3 * NB)
            # mo4[(slot,hr), m, i] = T[i, m] * s[m]  (tuT is m-major: packed)
            mo4 = ctpool.tile([P, NB * NHR * BD * BD], BF16, tag="mo4",
                              name=f"mo4_{rb}")
            nc.vector.tensor_tensor(
                bass.AP(mo4.tensor, mo4[:].offset,
                        [[NB * GR, P], [BD * BD, NB * NHR], [BD, BD], [1, BD]]),
                bass.AP(tub.tensor, tub[:].offset,
                        [[NB * TUP, P], [BD * JW, NB * NHR], [BD, BD], [1, BD]]),
                bass.AP(s_mi.tensor, s_mi[:].offset,
                        [[NB * GR, P], [BD * BD, NB * NHR], [BD, BD], [1, BD]]),
                ALU.mult)
            # sum over m: bf16 add-tree 8 -> 4 -> 2 -> 1 (i innermost, packed)
            c1t = ctpool.tile([P, NB * NHR * 4 * BD], BF16, tag="c1",
                              name=f"c1_{rb}")
            nc.vector.tensor_tensor(
                c1t[:],
                bass.AP(mo4.tensor, mo4[:].offset,
                        [[NB * GR, P], [BD * BD, NB * NHR], [1, 4 * BD]]),
                bass.AP(mo4.tensor, mo4[:].offset + 4 * BD,
                        [[NB * GR, P], [BD * BD, NB * NHR], [1, 4 * BD]]),
                ALU.add)
            c2t = ctpool.tile([P, NB * NHR * 2 * BD], BF16, tag="c2",
                              name=f"c2_{rb}")
            nc.vector.tensor_tensor(
                c2t[:],
                bass.AP(c1t.tensor, c1t[:].offset,
                        [[NB * NHR * 4 * BD, P], [4 * BD, NB * NHR], [1, 2 * BD]]),
                bass.AP(c1t.tensor, c1t[:].offset + 2 * BD,
                        [[NB * NHR * 4 * BD, P], [4 * BD, NB * NHR], [1, 2 * BD]]),
                ALU.add)
            c3t = ctpool.tile([P, NB * HRI], BF16, tag="c3", name=f"c3_{rb}")
            nc.vector.tensor_tensor(
                c3t[:],
                bass.AP(c2t.tensor, c2t[:].offset,
                        [[NB * NHR * 2 * BD, P], [2 * BD, NB * NHR], [1, BD]]),
                bass.AP(c2t.tensor, c2t[:].offset + BD,
                        [[NB * NHR * 2 * BD, P], [2 * BD, NB * NHR], [1, BD]]),
                ALU.add)
            # + u_r (tuT col m=8 at offset 64)
            nc.vector.tensor_tensor(
                bass.AP(s_outb.tensor, s_outb[:].offset + rb * HRI,
                        [[C * HRI, P], [1, NB * HRI]]),
                c3t[:],
                bass.AP(tub.tensor, tub[:].offset + BD * BD,
                        [[NB * TUP, P], [BD * JW, NB * NHR], [1, BD]]),
                ALU.add)
            # convert finished eighth to f32 on ACT + dump, overlapped with C
            if (rb + NB) % (2 * NB) == 0:
                qu = (rb + NB) // (2 * NB) - 1
                HC = 2 * NB * HRI
                s_out = scpool.tile([P, HC], F32, tag="sout",
                                    name=f"sout{qu}")
                nc.scalar.activation(s_out[:],
                                     s_outb[:, qu * HC:(qu + 1) * HC],
                                     AF.Identity)
                nc.sync.dma_start(
                    bass.AP(out, qu * HC, [[C * HRI, P], [1, HC]]),
                    s_out[:])

    nc.compile()
    return nc


# ---------------- host side ----------------

_NC_CACHE = {}


def _get_nc(TOK=SEQ, K=16):
    key = (TOK, K)
    if key not in _NC_CACHE:
        _NC_CACHE[key] = build_nc(TOK=TOK, K=K)
    return _NC_CACHE[key]


def prep_shared(W1, b1, W2, b2, V1, c1, V2, c2, a0):
    bf = ml_dtypes.bfloat16
    W2r = W2.reshape(H, BD, BD, HID)
    W2c = (W2r - W2r.mean(axis=1, keepdims=True)).reshape(H * BD * BD, HID)
    b2r = b2.reshape(H, BD, BD)
    b2c = (b2r - b2r.mean(axis=1, keepdims=True)).reshape(-1)
    shared = {
        "w1": np.ascontiguousarray(W1.T).astype(bf),
        "b1": np.asarray(b1).reshape(HID, 1).astype(np.float32),
        "v1": np.ascontiguousarray(V1.T).astype(bf),
        "c1": np.asarray(c1).reshape(EMB, 1).astype(np.float32),
    }
    halves = []
    for half in range(2):
        rsl = slice(half * NF, (half + 1) * NF)
        vsl = slice(half * VF, (half + 1) * VF)
        hsl = slice(half * HL, (half + 1) * HL)
        a0h = np.asarray(a0)[0, hsl]                       # [32, 8]
        a0p = a0h.reshape(BD, 4, BD).reshape(BD, 32)       # [ho, (hr, i)]
        halves.append({
            "w2": np.ascontiguousarray(W2c[rsl].T).astype(bf),
            "b2": b2c[rsl].reshape(1, NF).astype(bf),
            "v2": np.ascontiguousarray(V2[vsl].T).astype(bf),
            "c2": np.asarray(c2)[vsl].reshape(1, VF).astype(bf),
            "a0": a0p.astype(np.float32),
        })
    return shared, halves


def make_in_maps(x, W1, b1, W2, b2, V1, c1, V2, c2, a0):
    shared, halves = prep_shared(W1, b1, W2, b2, V1, c1, V2, c2, a0)
    bf = ml_dtypes.bfloat16
    in_maps = []
    for core in range(N_CORES):
        b, half = core // 2, core % 2
        m = dict(shared)
        m.update(halves[half])
        m["xT"] = np.ascontiguousarray(np.asarray(x)[b].T).astype(bf)
        in_maps.append(m)
    return in_maps


def kernel(x, W1, b1, W2, b2, V1, c1, V2, c2, a0):
    from concourse import bass_utils
    nc = _get_nc(SEQ)
    in_maps = make_in_maps(x, W1, b1, W2, b2, V1, c1, V2, c2, a0)
    res = bass_utils.run_bass_kernel_spmd(nc, in_maps, core_ids=list(range(N_CORES)))
    out = np.zeros((BS, SEQ, EMB), np.float32)
    K_, C_ = 16, SEQ // 16
    for core in range(N_CORES):
        b, half = core // 2, core % 2
        # [(c,ho), (r, hr, i)] -> [t = c*C + r, (ho, hr, i)]
        r = res.results[core]["out"].reshape(K_, 8, C_, 4, 8)
        r = r.transpose(0, 2, 1, 3, 4).reshape(SEQ, VF)
        out[b, :, half * VF:(half + 1) * VF] = r
    return out


# revision 77
# speedup vs baseline: 1.0007x; 1.0007x over previous
"""Trainium2 Bass kernel for nn_BlockModel_82678120448388.

Model: per (batch, head): 8x8 transition matrices from an MLP (normalized),
values from a second MLP, then a linear recurrence s_t = A_t s_{t-1} + v_t
over seq=2048.

Sharding: 8 cores = 4 batches x 2 head-halves (32 heads each). Weights
replicated / row-sliced on host; full inputs in, full output out.

Scan: chunked scan (K=16 chunks x C=128). Phase 1 composes the per-chunk
[T|u] prefix chain in bf16 (mult + add-tree on DVE, all operands packed for
the 2x mode), interleaved under the MLP matmuls, spilling each prefix to
DRAM. Phase B combines chunk totals. Phase C applies s_r = T_r s_init + u_r
from the spilled prefixes -- fully parallel, no dependency chain.
"""

import numpy as np
import ml_dtypes
from contextlib import ExitStack

import concourse.bass as bass
import concourse.bacc as bacc
import concourse.tile as tile
from concourse import mybir

F32 = mybir.dt.float32
BF16 = mybir.dt.bfloat16
AF = mybir.ActivationFunctionType
ALU = mybir.AluOpType

BS, SEQ, EMB, BD = 4, 2048, 512, 8
H = EMB // BD      # 64 global heads
HL = 32            # heads per core
NF = HL * BD * BD  # 2048 blk feats per core
VF = HL * BD       # 256 v feats per core
AVF = NF + VF      # 2304 combined A+v row
HID = EMB * BD     # 4096
P = 128
JW = BD + 1        # [T|u] column count

N_CORES = 8


def build_nc(TOK=SEQ, K=16, p1_steps=None, pc_steps=None, nq_steps=None):
    """Per-core Bass module. TOK tokens, K chunks (chunk len C=TOK//K)."""
    C = TOK // K
    QT = min(256, TOK)     # L1 token-chunk
    NQ = TOK // QT
    TPQ = QT // P          # tok-tiles per q
    NHO = P // K           # head-groups per chunk on partitions (8 for K=16)
    NHR = HL // NHO        # heads per group in free dim (4)
    HRI = NHR * BD         # 32
    TUP = NHR * BD * JW    # 288 = per-partition [T|u]^T size (hr, m9, i8)
    GR = NHR * BD * BD     # 256 = A feats per (ho) group
    GRV = GR + NHR * BD    # 288 = A+v feats per (ho) group

    assert TOK % QT == 0 and QT % P == 0 and P % K == 0

    nc = bacc.Bacc("TRN2", target_bir_lowering=False, debug=False)

    xT = nc.dram_tensor("xT", [EMB, TOK], BF16, kind="ExternalInput")
    w1 = nc.dram_tensor("w1", [EMB, HID], BF16, kind="ExternalInput")
    b1 = nc.dram_tensor("b1", [HID, 1], F32, kind="ExternalInput")
    w2 = nc.dram_tensor("w2", [HID, NF], BF16, kind="ExternalInput")
    b2 = nc.dram_tensor("b2", [1, NF], BF16, kind="ExternalInput")
    v1 = nc.dram_tensor("v1", [EMB, EMB], BF16, kind="ExternalInput")
    c1 = nc.dram_tensor("c1", [EMB, 1], F32, kind="ExternalInput")
    v2 = nc.dram_tensor("v2", [EMB, VF], BF16, kind="ExternalInput")
    c2 = nc.dram_tensor("c2", [1, VF], BF16, kind="ExternalInput")
    a0 = nc.dram_tensor("a0", [NHO, HRI], F32, kind="ExternalInput")
    # native scan layout [(c,ho), (r, hr, i)]; host reindexes to [t, vf]
    out = nc.dram_tensor("out", [P, C * HRI], F32, kind="ExternalOutput")

    # spilled per-token prefixes [T|u]^T, r-minor: [P, C, TUP]
    tu_dram = nc.dram_tensor("tu_scratch", [P, C * TUP], BF16)

    with ExitStack() as ctx:
        tc = ctx.enter_context(tile.TileContext(nc))
        cpool = ctx.enter_context(tc.tile_pool(name="consts", bufs=1))
        wpool = ctx.enter_context(tc.tile_pool(name="weights", bufs=1))
        xpool = ctx.enter_context(tc.tile_pool(name="xstream", bufs=2))
        hpool = ctx.enter_context(tc.tile_pool(name="hidden", bufs=2))
        w2pool = ctx.enter_context(tc.tile_pool(name="w2stream", bufs=4))
        l1ps = ctx.enter_context(tc.tile_pool(name="l1ps", bufs=2, space="PSUM"))
        l2ps = ctx.enter_context(tc.tile_pool(name="l2ps", bufs=2 * TPQ, space="PSUM"))
        vps = ctx.enter_context(tc.tile_pool(name="vps", bufs=2, space="PSUM"))
        blkpool = ctx.enter_context(tc.tile_pool(name="blk", bufs=TPQ + 1))
        abpool = ctx.enter_context(tc.tile_pool(name="ab", bufs=2))
        sqpool = ctx.enter_context(tc.tile_pool(name="sq", bufs=2))
        smpool = ctx.enter_context(tc.tile_pool(name="small", bufs=2))
        agpool = ctx.enter_context(tc.tile_pool(name="agather", bufs=5))
        mopool = ctx.enter_context(tc.tile_pool(name="multout", bufs=2))
        tupool = ctx.enter_context(tc.tile_pool(name="tu", bufs=4))
        tbpool = ctx.enter_context(tc.tile_pool(name="tub", bufs=3))
        ctpool = ctx.enter_context(tc.tile_pool(name="ctree", bufs=1))
        scpool = ctx.enter_context(tc.tile_pool(name="scan", bufs=1))

        # ---- constants / weights ----
        ones_s = cpool.tile([1, P], BF16, tag="ones")
        nc.vector.memset(ones_s[:], 1.0)
        b1_s = cpool.tile([P, HID // P], F32, tag="b1")
        nc.sync.dma_start(b1_s[:], b1[:].rearrange("(m p) one -> p (m one)", p=P))
        c1_s = cpool.tile([P, EMB // P], F32, tag="c1")
        nc.sync.dma_start(c1_s[:], c1[:].rearrange("(m p) one -> p (m one)", p=P))
        b2_s = cpool.tile([1, NF], BF16, tag="b2")
        nc.sync.dma_start(b2_s[:], b2[:])
        c2_s = cpool.tile([1, VF], BF16, tag="c2")
        nc.sync.dma_start(c2_s[:], c2[:])
        a0_s = cpool.tile([NHO, HRI], F32, tag="a0")
        nc.sync.dma_start(a0_s[:], a0[:])

        w1_s = wpool.tile([P, 4, HID], BF16, tag="w1")
        v1_s = wpool.tile([P, 4, EMB], BF16, tag="v1")
        v2_s = wpool.tile([P, 4, VF], BF16, tag="v2")

        def load_weights():
            # w1 in m-blocks so the first L1 matmul starts after one block
            for b in range(8):
                nc.sync.dma_start(
                    bass.AP(w1_s.tensor, w1_s[:].offset + b * 512,
                            [[4 * HID, P], [HID, 4], [1, 512]]),
                    bass.AP(w1, b * 512,
                            [[HID, P], [P * HID, 4], [1, 512]]))
            nc.sync.dma_start(v1_s[:], v1[:].rearrange("(k p) m -> p k m", p=P))
            nc.sync.dma_start(v2_s[:], v2[:].rearrange("(k p) n -> p k n", p=P))

        # ================= scan helpers =================
        # av_dram row tau*128 + c*8 + j holds token c*C + 8*tau + j.
        # Phase 1 partition = (chunk c, ho); tuT[(c,ho), (hr, m9, i8)] =
        # [T|u][i, m] (transposed storage; u col at m=8, offset 64+i).

        def rowbase(r):
            tau, j = r // 8, r % 8
            return tau * P + j

        tu_box = {}

        def g_av(r, ab):
            # SBUF->SBUF redistribution: src partitions c*8+j of ab(tau)
            j = r % 8
            ag = agpool.tile([P, GRV], BF16, tag="ag", name=f"ag{r}")
            nc.sync.dma_start(ag[:], bass.AP(
                ab.tensor, ab[:].offset + j * AVF,
                [[8 * AVF, K], [GRV, NHO], [1, GRV]]))
            return ag

        def spill_tu(r, tu):
            nc.sync.dma_start(
                bass.AP(tu_dram, r * TUP, [[C * TUP, P], [1, TUP]]),
                tu[:])

        def phase1_init(ag):
            # tuT_0[m, i] = A_0[i, m] ; ag A block row-major (hr, i, m)
            tu = tupool.tile([P, TUP], BF16, tag="tu", name="tu0")
            nc.vector.tensor_copy(
                bass.AP(tu.tensor, tu[:].offset,
                        [[TUP, P], [BD * JW, NHR], [BD, BD], [1, BD]]),
                bass.AP(ag.tensor, ag[:].offset,
                        [[GRV, P], [BD * BD, NHR], [1, BD], [BD, BD]]))
            nc.vector.tensor_copy(
                bass.AP(tu.tensor, tu[:].offset + BD * BD,
                        [[TUP, P], [BD * JW, NHR], [1, BD]]),
                bass.AP(ag.tensor, ag[:].offset + GR,
                        [[GRV, P], [BD, NHR], [1, BD]]))
            tu_box['tu'] = tu
            spill_tu(0, tu)

        def phase1_step(r, ag):
            # mo[hr, i, m, k] = A_r[i, k] * Told[k, m]; all packed bf16
            tu = tu_box['tu']
            mo = mopool.tile([P, NHR * BD * JW * BD], BF16, tag="mo",
                             name=f"mo{r}")
            # iteration (hr, i, m, k); per-operand dims merged to 3 free
            nc.vector.tensor_tensor(
                bass.AP(mo.tensor, mo[:].offset,
                        [[NHR * BD * JW * BD, P], [JW * BD, NHR * BD],
                         [BD, JW], [1, BD]]),
                bass.AP(ag.tensor, ag[:].offset,
                        [[GRV, P], [BD, NHR * BD], [0, JW], [1, BD]]),
                bass.AP(tu.tensor, tu[:].offset,
                        [[TUP, P], [BD * JW, NHR], [0, BD], [1, BD * JW]]),
                ALU.mult)
            # add-tree over k (innermost): 8 -> 4 -> 2 -> 1
            t1 = mopool.tile([P, NHR * BD * JW * 4], BF16, tag="t1",
                             name=f"t1_{r}")
            nc.vector.tensor_tensor(
                t1[:], bass.AP(mo.tensor, mo[:].offset,
                               [[NHR * BD * JW * BD, P], [BD, NHR * BD * JW],
                                [1, 4]]),
                bass.AP(mo.tensor, mo[:].offset + 4,
                        [[NHR * BD * JW * BD, P], [BD, NHR * BD * JW], [1, 4]]),
                ALU.add)
            t2 = mopool.tile([P, NHR * BD * JW * 2], BF16, tag="t2",
                             name=f"t2_{r}")
            nc.vector.tensor_tensor(
                t2[:], bass.AP(t1.tensor, t1[:].offset,
                               [[NHR * BD * JW * 4, P], [4, NHR * BD * JW],
                                [1, 2]]),
                bass.AP(t1.tensor, t1[:].offset + 2,
                        [[NHR * BD * JW * 4, P], [4, NHR * BD * JW], [1, 2]]),
                ALU.add)
            # final add writes transposed: tun[(hr, m, i)] = T_new[i, m]
            # t2 elem (hr, i, m, k) at hr*144 + i*18 + m*2 + k
            tun = tupool.tile([P, TUP], BF16, tag="tu", name=f"tu{r}")
            nc.vector.tensor_tensor(
                bass.AP(tun.tensor, tun[:].offset,
                        [[TUP, P], [BD * JW, NHR], [BD, JW], [1, BD]]),
                bass.AP(t2.tensor, t2[:].offset,
                        [[NHR * BD * JW * 2, P], [JW * BD * 2, NHR],
                         [2, JW], [JW * 2, BD]]),
                bass.AP(t2.tensor, t2[:].offset + 1,
                        [[NHR * BD * JW * 2, P], [JW * BD * 2, NHR],
                         [2, JW], [JW * 2, BD]]),
                ALU.add)
            # u += v_r  (u col at m=8: offset 64 + i; v in ag at GR + hr*8+i)
            nc.vector.tensor_tensor(
                bass.AP(tun.tensor, tun[:].offset + BD * BD,
                        [[TUP, P], [BD * JW, NHR], [1, BD]]),
                bass.AP(tun.tensor, tun[:].offset + BD * BD,
                        [[TUP, P], [BD * JW, NHR], [1, BD]]),
                bass.AP(ag.tensor, ag[:].offset + GR,
                        [[GRV, P], [BD, NHR], [1, BD]]),
                ALU.add)
            tu_box['tu'] = tun
            spill_tu(r, tun)

        # ================= stage A (+ interleaved phase 1) =================
        def load_x(q):
            RPQ = TPQ * 8
            xq = xpool.tile([P, 4, QT], BF16, tag="xq", name=f"xq{q}")
            for ttq in range(TPQ):
                # tile tau = q*TPQ+ttq: tokens c*C + 8*tau + j, col order (c, j)
                for k in range(4):
                    nc.sync.dma_start(
                        xq[:, k, bass.ts(ttq, P)],
                        bass.AP(xT, k * P * TOK + q * RPQ + ttq * 8,
                                [[TOK, P], [C, K], [1, 8]]))
            return xq

        xq0 = load_x(0)
        load_weights()

        def stage_l1(q, xq=None):
            """first-layer MLPs for q; returns (hid_t, hv_t)."""
            if xq is None:
                xq = load_x(q)
            hid_t = hpool.tile([P, HID // P, QT], BF16, tag="hid",
                               name=f"hid{q}")
            for m in range(HID // P):
                ps = l1ps.tile([P, QT], F32, tag="l1")
                for k in range(4):
                    nc.tensor.matmul(ps[:], w1_s[:, k, bass.ts(m, P)],
                                     xq[:, k, :], start=(k == 0), stop=(k == 3))
                nc.scalar.activation(hid_t[:, m, :], ps[:], AF.Relu,
                                     bias=b1_s[:, m:m + 1])
            hv_t = hpool.tile([P, 4, QT], BF16, tag="hv", name=f"hv{q}")
            for m in range(4):
                ps = l1ps.tile([P, QT], F32, tag="l1")
                for k in range(4):
                    nc.tensor.matmul(ps[:], v1_s[:, k, bass.ts(m, P)], xq[:, k, :],
                                     start=(k == 0), stop=(k == 3))
                nc.scalar.activation(hv_t[:, m, :], ps[:], AF.Relu,
                                     bias=c1_s[:, m:m + 1])
            return hid_t, hv_t

        l1_box = {0: stage_l1(0, xq0)}
        for q in range(NQ if nq_steps is None else nq_steps):
            hid_t, hv_t = l1_box.pop(q)

            # ---- L2: token-major blk, W2 streamed 4 k-slices per DMA ----
            blks = [blkpool.tile([P, NF], BF16, tag="blk", name=f"blk{q}_{i}") for i in range(TPQ)]
            for n in range(NF // 512):
                pss = [l2ps.tile([P, 512], F32, tag="l2", name=f"l2ps{q}_{n}_{i}") for i in range(TPQ)]
                for ttq in range(TPQ):
                    nc.tensor.matmul(pss[ttq][:], ones_s[:1, :],
                                     b2_s[:1, bass.ts(n, 512)], start=True, stop=False)
                for kk in range(HID // P // 4):
                    w2s = w2pool.tile([P, 4, 512], BF16, tag="w2s")
                    nc.sync.dma_start(
                        w2s[:], w2[bass.ds(kk * 4 * P, 4 * P), bass.ts(n, 512)]
                        .rearrange("(k p) n -> p k n", p=P))
                    for k4 in range(4):
                        k = kk * 4 + k4
                        for ttq in range(TPQ):
                            nc.tensor.matmul(pss[ttq][:], hid_t[:, k, bass.ts(ttq, P)],
                                             w2s[:, k4, :], start=False,
                                             stop=(k == HID // P - 1))
                for ttq in range(TPQ):
                    nc.scalar.activation(blks[ttq][:, bass.ts(n, 512)], pss[ttq][:],
                                         AF.Identity)

            # hoist next q's L1 so its relus run before this q's norm chain
            if q + 1 < NQ:
                l1_box[q + 1] = stage_l1(q + 1)

            # ---- v2 + normalization; ACT calls batched by function ----
            abs_ = [abpool.tile([P, AVF], BF16, tag="ab", name=f"ab{q}_{i}")
                    for i in range(TPQ)]
            for ttq in range(TPQ):
                psv = vps.tile([P, VF], F32, tag="v", name=f"v{q}_{ttq}")
                nc.tensor.matmul(psv[:], ones_s[:1, :], c2_s[:1, :],
                                 start=True, stop=False)
                for k in range(4):
                    nc.tensor.matmul(psv[:], hv_t[:, k, bass.ts(ttq, P)],
                                     v2_s[:, k, :], start=False, stop=(k == 3))
                # v into ab at (ho*288 + 256 + hr*8 + i)
                nc.scalar.activation(
                    bass.AP(abs_[ttq].tensor, abs_[ttq][:].offset + GR,
                            [[AVF, P], [GRV, NHO], [1, NHR * BD]]),
                    psv[:], AF.Identity)
            pwfs = [sqpool.tile([P, NF], F32, tag="pwf", name=f"pwf{q}_{i}")
                    for i in range(TPQ)]
            for ttq in range(TPQ):
                # square on DVE (bf16 packed, 2x) to shorten the ACT chain;
                # staged in ab's A-region (overwritten by the A-write below)
                nc.vector.tensor_tensor(
                    bass.AP(abs_[ttq].tensor, abs_[ttq][:].offset,
                            [[AVF, P], [GRV, NHO], [1, GR]]),
                    bass.AP(blks[ttq].tensor, blks[ttq][:].offset,
                            [[NF, P], [GR, NHO], [1, GR]]),
                    bass.AP(blks[ttq].tensor, blks[ttq][:].offset,
                            [[NF, P], [GR, NHO], [1, GR]]),
                    ALU.mult)
            for ttq in range(TPQ):
                nc.scalar.activation(
                    pwfs[ttq][:],
                    bass.AP(abs_[ttq].tensor, abs_[ttq][:].offset,
                            [[AVF, P], [GRV, NHO], [1, GR]]),
                    AF.Ln)
            for ttq in range(TPQ):
                nc.scalar.activation(pwfs[ttq][:], pwfs[ttq][:], AF.Exp,
                                     scale=0.6)
            dms = []
            for ttq in range(TPQ):
                pw = pwfs[ttq]
                # sum over i: feat = h*64 + i*8 + k -> dims [p, h, k, i]
                pst = smpool.tile([P, HL * BD], F32, tag="pst",
                                  name=f"pst{q}_{ttq}")
                nc.vector.tensor_reduce(
                    pst[:].rearrange("p (h k) -> p h k", h=HL, k=BD),
                    bass.AP(pw.tensor, pw[:].offset,
                            [[NF, P], [64, HL], [1, BD], [8, BD]]),
                    axis=mybir.AxisListType.X, op=ALU.add)
                # max_k (sum)^(1/1.2) == (max_k sum)^(1/1.2): max in linear
                dm = smpool.tile([P, HL], F32, tag="dm", name=f"dm{q}_{ttq}")
                nc.vector.tensor_reduce(
                    dm[:].rearrange("p (h one) -> p h one", h=HL, one=1),
                    pst[:].rearrange("p (h k) -> p h k", h=HL, k=BD),
                    axis=mybir.AxisListType.X, op=ALU.max)
                dms.append(dm)
            rcs = []
            for ttq in range(TPQ):
                nc.scalar.activation(dms[ttq][:], dms[ttq][:], AF.Ln)
            for ttq in range(TPQ):
                rc = smpool.tile([P, HL], F32, tag="rc", name=f"rc{q}_{ttq}")
                # rc = dm^(-1/1.2)
                nc.scalar.activation(rc[:], dms[ttq][:], AF.Exp,
                                     scale=-1.0 / 1.2)
                rcs.append(rc)
            # ---- per tile: A-write + that tile's 8 phase-1 steps ----
            for ttq in range(TPQ):
                tt = q * TPQ + ttq
                ab, blk, rc = abs_[ttq], blks[ttq], rcs[ttq]
                # A = blk * rc (broadcast over i, k): row-major into ab
                nc.vector.tensor_tensor(
                    bass.AP(ab.tensor, ab[:].offset,
                            [[AVF, P], [GRV, NHO], [1, GR]]),
                    bass.AP(blk.tensor, blk[:].offset,
                            [[NF, P], [GR, NHO], [1, GR]]),
                    bass.AP(rc.tensor, rc[:].offset,
                            [[HL, P], [NHR, NHO], [1, NHR], [0, BD * BD]]),
                    ALU.mult)
                for r in range(tt * 8, tt * 8 + 8):
                    if p1_steps is not None and r >= p1_steps:
                        continue
                    ag = g_av(r, ab)
                    if r == 0:
                        phase1_init(ag)
                    else:
                        phase1_step(r, ag)

        # preload first phase-C prefix batches while phase B runs
        NB = 8
        def load_tub(rb):
            tub = tbpool.tile([P, NB, TUP], BF16, tag="tu4", name=f"tu4_{rb}")
            nc.sync.dma_start(tub[:], bass.AP(
                tu_dram, rb * TUP,
                [[C * TUP, P], [1, NB * TUP]]))
            return tub
        tub_q = {rb: load_tub(rb) for rb in (0, NB, 2 * NB)}

        # ---- phase B: chunk-level combine (on partitions 0:NHO) ----
        TUPK = K * TUP
        tu2 = scpool.tile([NHO, TUPK], BF16, tag="tu2")
        nc.sync.dma_start(
            bass.AP(tu2.tensor, tu2[:].offset,
                    [[TUPK, NHO], [TUP, K], [1, TUP]]),
            bass.AP(tu_dram, (C - 1) * TUP,
                    [[C * TUP, NHO], [NHO * C * TUP, K], [1, TUP]]))
        s_seq = scpool.tile([NHO, (K + 1) * HRI], F32, tag="sseq")
        nc.vector.tensor_copy(s_seq[:, 0:HRI], a0_s[:])
        for c in range(K):
            # mo3[hr, i, k] = Tc[i, k] * s[k]; Tc[i,k] at tuT offset k*8+i
            mo3 = mopool.tile([NHO, HRI * BD], F32, tag="mo3")
            nc.vector.tensor_tensor(
                bass.AP(mo3.tensor, mo3[:].offset,
                        [[HRI * BD, NHO], [BD * BD, NHR], [1, BD], [BD, BD]]),
                bass.AP(tu2.tensor, tu2[:].offset + c * TUP,
                        [[TUPK, NHO], [BD * JW, NHR], [1, BD], [BD, BD]]),
                bass.AP(s_seq.tensor, s_seq[:].offset + c * HRI,
                        [[(K + 1) * HRI, NHO], [BD, NHR], [0, BD], [1, BD]]),
                ALU.mult)
            sn3 = smpool.tile([NHO, HRI], F32, tag="sn3")
            nc.vector.tensor_reduce(
                bass.AP(sn3.tensor, sn3[:].offset, [[HRI, NHO], [1, HRI]]),
                bass.AP(mo3.tensor, mo3[:].offset,
                        [[HRI * BD, NHO], [BD, HRI], [1, BD]]),
                axis=mybir.AxisListType.X, op=ALU.add)
            nc.vector.tensor_tensor(
                bass.AP(s_seq.tensor, s_seq[:].offset + (c + 1) * HRI,
                        [[(K + 1) * HRI, NHO], [BD, NHR], [1, BD]]),
                bass.AP(sn3.tensor, sn3[:].offset, [[HRI, NHO], [BD, NHR], [1, BD]]),
                bass.AP(tu2.tensor, tu2[:].offset + c * TUP + BD * BD,
                        [[TUPK, NHO], [BD * JW, NHR], [1, BD]]),
                ALU.add)
        # relayout chunk-start states -> s_init [(c,ho), (hr,k)]
        s_init = scpool.tile([P, HRI], F32, tag="sinit")
        for c in range(K):
            nc.sync.dma_start(s_init[c * NHO:(c + 1) * NHO, :],
                              s_seq[:, c * HRI:(c + 1) * HRI])

        # ---- phase C: s_r = T_r s_init + u_r from spilled prefixes ----
        # column-major: s_r[i] = sum_m T[i, m]-col * s[m]; all bf16 packed
        s_outb = scpool.tile([P, C * HRI], BF16, tag="soutb")
        # s_mi[(slot,hr), m, i] = s_init[hr, m] replicated over slot, i
        s_mi = scpool.tile([P, NB * NHR * BD * BD], BF16, tag="smi")
        for slot in range(NB):
            nc.vector.tensor_copy(
                bass.AP(s_mi.tensor, s_mi[:].offset + slot * GR,
                        [[NB * GR, P], [BD * BD, NHR], [BD, BD], [1, BD]]),
                bass.AP(s_init.tensor, s_init[:].offset,
                        [[HRI, P], [BD, NHR], [1, BD], [0, BD]]))
        for rb in range(0, C if pc_steps is None else pc_steps, NB):
            tub = tub_q.pop(rb)
            if rb + 3 * NB < C:
                tub_q[rb + 3 * NB] = load_tub(rb + 3 * NB)
            # mo4[(slot,hr), m, i] = T[i, m] * s[m]  (tuT is m-major: packed)
            mo4 = ctpool.tile([P, NB * NHR * BD * BD], BF16, tag="mo4",
                              name=f"mo4_{rb}")
            nc.vector.tensor_tensor(
                bass.AP(mo4.tensor, mo4[:].offset,
                        [[NB * GR, P], [BD * BD, NB * NHR], [BD, BD], [1, BD]]),
                bass.AP(tub.tensor, tub[:].offset,
                        [[NB * TUP, P], [BD * JW, NB * NHR], [BD, BD], [1, BD]]),
                bass.AP(s_mi.tensor, s_mi[:].offset,
                        [[NB * GR, P], [BD * BD, NB * NHR], [BD, BD], [1, BD]]),
                ALU.mult)
            # sum over m: bf16 add-tree 8 -> 4 -> 2 -> 1 (i innermost, packed)
            c1t = ctpool.tile([P, NB * NHR * 4 * BD], BF16, tag="c1",
                              name=f"c1_{rb}")
            nc.vector.tensor_tensor(
                c1t[:],
                bass.AP(mo4.tensor, mo4[:].offset,
                        [[NB * GR, P], [BD * BD, NB * NHR], [1, 4 * BD]]),
                bass.AP(mo4.tensor, mo4[:].offset + 4 * BD,
                        [[NB * GR, P], [BD * BD, NB * NHR], [1, 4 * BD]]),
                ALU.add)
            c2t = ctpool.tile([P, NB * NHR * 2 * BD], BF16, tag="c2",
                              name=f"c2_{rb}")
            nc.vector.tensor_tensor(
                c2t[:],
                bass.AP(c1t.tensor, c1t[:].offset,
                        [[NB * NHR * 4 * BD, P], [4 * BD, NB * NHR], [1, 2 * BD]]),
                bass.AP(c1t.tensor, c1t[:].offset + 2 * BD,
                        [[NB * NHR * 4 * BD, P], [4 * BD, NB * NHR], [1, 2 * BD]]),
                ALU.add)
            c3t = ctpool.tile([P, NB * HRI], BF16, tag="c3", name=f"c3_{rb}")
            nc.vector.tensor_tensor(
                c3t[:],
                bass.AP(c2t.tensor, c2t[:].offset,
                        [[NB * NHR * 2 * BD, P], [2 * BD, NB * NHR], [1, BD]]),
                bass.AP(c2t.tensor, c2t[:].offset + BD,
                        [[NB * NHR * 2 * BD, P], [2 * BD, NB * NHR], [1, BD]]),
                ALU.add)
            # + u_r (tuT col m=8 at offset 64)
            nc.vector.tensor_tensor(
                bass.AP(s_outb.tensor, s_outb[:].offset + rb * HRI,
                        [[C * HRI, P], [1, NB * HRI]]),
                c3t[:],
                bass.AP(tub.tensor, tub[:].offset + BD * BD,
                        [[NB * TUP, P], [BD * JW, NB * NHR], [1, BD]]),
                ALU.add)
            # convert finished eighth to f32 on ACT + dump, overlapped with C
            if (rb + NB) % (2 * NB) == 0:
                qu = (rb + NB) // (2 * NB) - 1
                HC = 2 * NB * HRI
                s_out = scpool.tile([P, HC], F32, tag="sout",
                                    name=f"sout{qu}")
                nc.scalar.activation(s_out[:],
                                     s_outb[:, qu * HC:(qu + 1) * HC],
                                     AF.Identity)
                nc.sync.dma_start(
                    bass.AP(out, qu * HC, [[C * HRI, P], [1, HC]]),
                    s_out[:])

    nc.compile()
    return nc


# ---------------- host side ----------------

_NC_CACHE = {}


def _get_nc(TOK=SEQ, K=16):
    key = (TOK, K)
    if key not in _NC_CACHE:
        _NC_CACHE[key] = build_nc(TOK=TOK, K=K)
    return _NC_CACHE[key]


def prep_shared(W1, b1, W2, b2, V1, c1, V2, c2, a0):
    bf = ml_dtypes.bfloat16
    W2r = W2.reshape(H, BD, BD, HID)
    W2c = (W2r - W2r.mean(axis=1, keepdims=True)).reshape(H * BD * BD, HID)
    b2r = b2.reshape(H, BD, BD)
    b2c = (b2r - b2r.mean(axis=1, keepdims=True)).reshape(-1)
    shared = {
        "w1": np.ascontiguousarray(W1.T).astype(bf),
        "b1": np.asarray(b1).reshape(HID, 1).astype(np.float32),
        "v1": np.ascontiguousarray(V1.T).astype(bf),
        "c1": np.asarray(c1).reshape(EMB, 1).astype(np.float32),
    }
    halves = []
    for half in range(2):
        rsl = slice(half * NF, (half + 1) * NF)
        vsl = slice(half * VF, (half + 1) * VF)
        hsl = slice(half * HL, (half + 1) * HL)
        a0h = np.asarray(a0)[0, hsl]                       # [32, 8]
        a0p = a0h.reshape(BD, 4, BD).reshape(BD, 32)       # [ho, (hr, i)]
        halves.append({
            "w2": np.ascontiguousarray(W2c[rsl].T).astype(bf),
            "b2": b2c[rsl].reshape(1, NF).astype(bf),
            "v2": np.ascontiguousarray(V2[vsl].T).astype(bf),
            "c2": np.asarray(c2)[vsl].reshape(1, VF).astype(bf),
            "a0": a0p.astype(np.float32),
        })
    return shared, halves


def make_in_maps(x, W1, b1, W2, b2, V1, c1, V2, c2, a0):
    shared, halves = prep_shared(W1, b1, W2, b2, V1, c1, V2, c2, a0)
    bf = ml_dtypes.bfloat16
    in_maps = []
    for core in range(N_CORES):
        b, half = core // 2, core % 2
        m = dict(shared)
        m.update(halves[half])
        m["xT"] = np.ascontiguousarray(np.asarray(x)[b].T).astype(bf)
        in_maps.append(m)
    return in_maps


def kernel(x, W1, b1, W2, b2, V1, c1, V2, c2, a0):
    from concourse import bass_utils
    nc = _get_nc(SEQ)
    in_maps = make_in_maps(x, W1, b1, W2, b2, V1, c1, V2, c2, a0)
    res = bass_utils.run_bass_kernel_spmd(nc, in_maps, core_ids=list(range(N_CORES)))
    out = np.zeros((BS, SEQ, EMB), np.float32)
    K_, C_ = 16, SEQ // 16
    for core in range(N_CORES):
        b, half = core // 2, core % 2
        # [(c,ho), (r, hr, i)] -> [t = c*C + r, (ho, hr, i)]
        r = res.results[core]["out"].reshape(K_, 8, C_, 4, 8)
        r = r.transpose(0, 2, 1, 3, 4).reshape(SEQ, VF)
        out[b, :, half * VF:(half + 1) * VF] = r
    return out
